# revision 26
# baseline (speedup 1.0000x reference)
"""Trainium2 Bass kernel for nn_LlamaAttention_6588479832091.

Math notes:
  - The reference attention contracts q and k at the SAME sequence position
    (scores = einsum('bshd,bstd->bsht', q, k)), and RoPE applies the same
    orthogonal transform to q and k at equal positions, so RoPE cancels
    exactly: (P R q)·(P R k) = q·k.  v and the output path never see RoPE.
    The kernel therefore computes: q/k/v projections, per-token 16x16
    cross-head softmax attention, and the output projection.
  - Sharding: data-parallel over the 16384 tokens -> 2048 tokens per core,
    weights replicated.  No collectives.
  - All matmuls run in bf16 (1 cycle/row on the PE; fp32 would be 4) with
    fp32 PSUM accumulation.  End-to-end rel err ~5e-3, tolerance is 2e-2.
  - Fully fused per-512-token-chunk pipeline: the q/k/v projection psums are
    evacuated DIRECTLY into the attention's group-packed SBUF layout (no
    DRAM roundtrip, no staging loads).  Weight slabs are re-streamed per
    chunk instead (DMA is far below the PE roofline).  Emission order
    proj(0), proj(1), A(0), proj(2), A(1), proj(3), A(2), A(3) keeps the
    PE busy across chunk boundaries.
  - Attention softmax work is spread over DVE/ACT/Pool so no single engine
    exceeds the PE's per-macro cadence: exp on ACT, mask-mul + recip +
    normalize on DVE, v-transpose evac on Pool, attn-transpose evac split
    ACT/Pool.  Mask is multiplicative (0/1) applied to exp(scores); scores
    are O(few) so exp never overflows.

Layouts (host-prepared, all partition-first, bf16):
  xt   [128, 4, 8192]   xt[p, t, kt*512+i] = x_shard[t*512+i, kt*128+p]
  wq4  [128, 16, 2048]  wq4[p, mt, kt*128+j] = wq[mt*128+j, kt*128+p]/sqrt(128)
  wk4, wv4: same layout as wq4 (wk, wv unscaled)
  wo4  [128, 16, 2048]  wo4[p, rt, kt*128+j] = wo[rt*128+j, kt*128+p]
  maskd [128, 512]      1 where p%8 == n%8 else 0 (tiled x4 groups)
  identd [128, 128]     identity
  otb  [128, 16, 2048]  otb[p, rt, t] = out_shard[t, rt*128+p]   (output)
"""
import sys

for _p in ("/opt/trn_rl_repo", "/root/.axon_site/_ro/trn_rl_repo"):
    if _p not in sys.path:
        sys.path.insert(0, _p)

import numpy as np

T_CORE = 2048      # tokens per core
N_CORES = 8
H = 16             # heads
HD = 128           # head dim
HIDDEN = 2048
CT = HIDDEN // 128  # 16 contraction tiles
TCH = 512          # tokens per fused chunk
NTCH = T_CORE // TCH  # 4 chunks
GRP = 8            # tokens per attention group
NG = TCH // GRP    # 64 groups per chunk
MAC = 32           # tokens per macro (4 groups)
NMAC = TCH // MAC  # 16 macros per chunk

_CACHED = {}


def _build():
    import concourse.mybir as mybir
    import concourse.tile as tile
    import concourse.bacc as bacc

    f32 = mybir.dt.float32
    bf16 = mybir.dt.bfloat16
    EXP = mybir.ActivationFunctionType.Exp

    nc = bacc.Bacc("TRN2", target_bir_lowering=False, debug=False)

    xt = nc.declare_dram_parameter("xt", [128, NTCH, CT * TCH], bf16, isOutput=False)
    wq4 = nc.declare_dram_parameter("wq4", [128, H, CT * 128], bf16, isOutput=False)
    wk4 = nc.declare_dram_parameter("wk4", [128, H, CT * 128], bf16, isOutput=False)
    wv4 = nc.declare_dram_parameter("wv4", [128, H, CT * 128], bf16, isOutput=False)
    wo4 = nc.declare_dram_parameter("wo4", [128, CT, CT * 128], bf16, isOutput=False)
    maskd = nc.declare_dram_parameter("maskd", [128, 512], bf16, isOutput=False)
    identd = nc.declare_dram_parameter("identd", [128, 128], bf16, isOutput=False)
    otb = nc.declare_dram_parameter("otb", [128, CT, T_CORE], bf16, isOutput=True)

    with tile.TileContext(nc) as tc:
        with tc.tile_pool(name="io", bufs=1) as io, \
             tc.tile_pool(name="wp", bufs=1) as wp, \
             tc.tile_pool(name="xp", bufs=1) as xp, \
             tc.tile_pool(name="qk", bufs=1) as qkp, \
             tc.tile_pool(name="aw", bufs=1) as aw, \
             tc.tile_pool(name="ps", bufs=1, space="PSUM") as psp:

            mask_sb = io.tile([128, 512], bf16, name="masksb")
            ident_sb = io.tile([128, 128], bf16, name="identsb")
            ones_sb = io.tile([128, 1], bf16, name="onessb")
            nc.gpsimd.memset(ones_sb[:], 1.0)

            def make_proj(t):
                """q/k/v projections for 512 tokens, evacuated straight into
                the attention's packed layout [128=d, group, (h, tj)].
                Returns (pk dict, generator yielding after each psum-group)."""
                x_sb = xp.tile([128, CT * TCH], bf16, tag="x", bufs=2, name="xsb")
                # x off the sync queue (parallel with slab loads); chunk 0
                # alternates ACT/Pool queues so per-DMA overheads overlap
                for piece in range(4):
                    sl = slice(piece * 4 * TCH, (piece + 1) * 4 * TCH)
                    eng = nc.gpsimd if (t == 0 and piece % 2) else nc.scalar
                    eng.dma_start(x_sb[:, sl], xt[:, t, sl])
                pk = {}
                for wname in ("q", "k", "v"):
                    pk[wname] = qkp.tile([128, NG, 128], bf16, tag=f"{wname}pk",
                                         bufs=2, name=f"{wname}pk")

                def gen():
                    for wname, wsrc in (("q", wq4), ("k", wk4), ("v", wv4)):
                        dst = pk[wname]
                        for mt2 in range(H // 2):
                            # two head-slabs per DMA: halves the DMA count
                            wslab = wp.tile([128, 2, CT * 128], bf16,
                                            tag="wslab", bufs=2, name="wslab")
                            if t == 0 and wname == "q" and mt2 == 0:
                                # two singles so the very first matmul group
                                # waits on half the transfer
                                nc.sync.dma_start(wslab[:, 0, :], wsrc[:, 0, :])
                                nc.sync.dma_start(wslab[:, 1, :], wsrc[:, 1, :])
                            else:
                                nc.sync.dma_start(
                                    wslab[:], wsrc[:, 2 * mt2:2 * mt2 + 2, :])
                            for j in range(2):
                                mt = 2 * mt2 + j
                                pp = psp.tile([128, TCH], f32, tag="big",
                                              bufs=2, name="pp")
                                for kt in range(CT):
                                    nc.tensor.matmul(
                                        pp[:],
                                        wslab[:, j, kt * 128:(kt + 1) * 128],
                                        x_sb[:, kt * TCH:(kt + 1) * TCH],
                                        start=(kt == 0), stop=(kt == CT - 1))
                                # v-evacs on ACT to relieve the DVE queue
                                ev_dst = dst[:, :, mt * GRP:(mt + 1) * GRP]
                                ev_src = pp[:].rearrange(
                                    "p (g tj) -> p g tj", tj=GRP)
                                if wname == "v":
                                    nc.scalar.copy(ev_dst, ev_src)
                                else:
                                    nc.vector.tensor_copy(ev_dst, ev_src)
                                yield
                return pk, gen()

            def make_attn(t, pk):
                """Cross-head attention macros for one chunk; emitted
                interleaved into PE-heavy windows so the softmax's DVE/ACT/
                Pool ops never outrun the PE. Returns (at tile, generator)."""
                qpk, kpk, vpk = pk["q"], pk["k"], pk["v"]
                at = aw.tile([128, CT, TCH], bf16, tag="at", bufs=2, name="atsb")
                st = {}

                def stage1(m):
                    ps_s = psp.tile([128, 512], f32, tag="s", bufs=2, name="ps_s")
                    for i in range(4):
                        g = 4 * m + i
                        nc.tensor.matmul(ps_s[:, i * 128:(i + 1) * 128],
                                         kpk[:, g, :], qpk[:, g, :],
                                         start=True, stop=True)
                    wt0 = aw.tile([128, 512], bf16, tag="wt0", bufs=3, name="wt0")
                    nc.scalar.activation(wt0[:], ps_s[:], EXP)
                    st[("wt0", m)] = wt0

                def stage1b(m):
                    # mask on Pool (SBUF-only engine) to offload DVE/ACT
                    wt0 = st.pop(("wt0", m))
                    wt = aw.tile([128, 512], bf16, tag="wt", bufs=3, name="wt")
                    nc.gpsimd.tensor_mul(wt[:], wt0[:], mask_sb[:])
                    st[("wt", m)] = wt

                def stage2(m):
                    wt = st[("wt", m)]
                    zt = psp.tile([128, TCH], f32, tag="big", bufs=2, name="zt")
                    for i in range(4):
                        nc.tensor.matmul(zt[:, i:i + 1],
                                         wt[:, i * 128:(i + 1) * 128], ones_sb[:],
                                         start=True, stop=True)
                    rz = aw.tile([128, 4], f32, tag="rz", bufs=3, name="rz")
                    nc.vector.reciprocal(rz[:], zt[:, :4])
                    st[("rz", m)] = rz
                    ps_v = psp.tile([128, 512], bf16, tag="v", bufs=1, name="ps_v")
                    for i in range(4):
                        g = 4 * m + i
                        nc.tensor.transpose(ps_v[:, i * 128:(i + 1) * 128],
                                            vpk[:, g, :], ident_sb[:])
                    vp = aw.tile([128, 512], bf16, tag="vp", bufs=3, name="vp")
                    nc.vector.tensor_copy(vp[:], ps_v[:])
                    st[("vp", m)] = vp

                def stage3(m):
                    wt = st.pop(("wt", m))
                    vp = st.pop(("vp", m))
                    rz = st.pop(("rz", m))
                    ps_at = psp.tile([128, 512], f32, tag="pat", bufs=2,
                                     name="ps_at")
                    for i in range(4):
                        nc.tensor.matmul(ps_at[:, i * 128:(i + 1) * 128],
                                         wt[:, i * 128:(i + 1) * 128],
                                         vp[:, i * 128:(i + 1) * 128],
                                         start=True, stop=True)
                    an = aw.tile([128, 512], bf16, tag="an", bufs=3, name="an")
                    nc.vector.tensor_mul(
                        an[:].rearrange("p (g c) -> p g c", g=4),
                        ps_at[:].rearrange("p (g c) -> p g c", g=4),
                        rz[:].broadcast_to((128, 4, 128)))
                    st[("an", m)] = an

                def stage4(m):
                    an = st.pop(("an", m))
                    ps_aT = psp.tile([128, 512], bf16, tag="aT", bufs=1,
                                     name="ps_aT")
                    for i in range(4):
                        nc.tensor.transpose(ps_aT[:, i * 128:(i + 1) * 128],
                                            an[:, i * 128:(i + 1) * 128],
                                            ident_sb[:])
                    # evac to at[d, h, tok] on ACT
                    nc.scalar.copy(
                        at[:, :, m * MAC:(m + 1) * MAC].rearrange(
                            "p h (g ti) -> p g h ti", ti=GRP),
                        ps_aT[:].rearrange(
                            "p (g h ti) -> p g h ti", g=4, h=H))

                def gen():
                    for m in range(NMAC + 4):
                        if m < NMAC:
                            stage1(m)
                        if 1 <= m <= NMAC:
                            stage1b(m - 1)
                        if 2 <= m <= NMAC + 1:
                            stage2(m - 2)
                        if 3 <= m <= NMAC + 2:
                            stage3(m - 3)
                        if 4 <= m <= NMAC + 3:
                            stage4(m - 4)
                        yield
                return at, gen()

            def make_oproj(t, at):
                """Output projection generator, one yield per rt group."""
                def gen():
                    for rt2 in range(CT // 2):
                        woslab = wp.tile([128, 2, CT * 128], bf16,
                                         tag="woslab", bufs=2, name="woslab")
                        nc.sync.dma_start(
                            woslab[:], wo4[:, 2 * rt2:2 * rt2 + 2, :])
                        for j in range(2):
                            rt = 2 * rt2 + j
                            po = psp.tile([128, TCH], f32, tag="big", bufs=2,
                                          name="po")
                            for kt in range(CT):
                                nc.tensor.matmul(
                                    po[:],
                                    woslab[:, j, kt * 128:(kt + 1) * 128],
                                    at[:, kt, :],
                                    start=(kt == 0), stop=(kt == CT - 1))
                            oev = aw.tile([128, TCH], bf16, tag="oev", bufs=2,
                                          name="oev")
                            nc.vector.tensor_copy(oev[:], po[:])
                            # last chunk: store via HWDGE (sync) — lower
                            # latency than SWDGE desc-gen, shortens the tail
                            eng = nc.sync if t == NTCH - 1 else nc.gpsimd
                            eng.dma_start(
                                otb[:, rt, t * TCH:(t + 1) * TCH], oev[:])
                            yield
                return gen()

            def interleave(gen_a, na, gen_b, nb):
                """Emit gen_a's units with gen_b's rate-matched in between."""
                done_b = 0
                for i in range(na):
                    next(gen_a)
                    want = min(nb, (i + 1) * nb // na + 1)
                    while done_b < want:
                        next(gen_b)
                        done_b += 1
                for _ in gen_a:
                    pass
                for _ in gen_b:
                    pass

            def drain(g):
                for _ in g:
                    pass

            # schedule: P0; P1(+)A0; O0(+)A1; P2; O1(+)A2; P3; O2(+)A3; O3
            pk0, pg0 = make_proj(0)
            # mask/ident after chunk0's x pieces on the ACT queue (only
            # needed once attention starts)
            nc.scalar.dma_start(mask_sb[:], maskd[:])
            nc.scalar.dma_start(ident_sb[:], identd[:])
            drain(pg0)
            pk1, pg1 = make_proj(1)
            at0, ag0 = make_attn(0, pk0)
            interleave(pg1, 48, ag0, NMAC + 4)
            og0 = make_oproj(0, at0)
            at1, ag1 = make_attn(1, pk1)
            interleave(og0, CT, ag1, NMAC + 4)
            pk2, pg2 = make_proj(2)
            drain(pg2)
            og1 = make_oproj(1, at1)
            at2, ag2 = make_attn(2, pk2)
            interleave(og1, CT, ag2, NMAC + 4)
            pk3, pg3 = make_proj(3)
            drain(pg3)
            og2 = make_oproj(2, at2)
            at3, ag3 = make_attn(3, pk3)
            interleave(og2, CT, ag3, NMAC + 4)
            og3 = make_oproj(3, at3)
            drain(og3)

    nc.compile()
    return nc


def _host_prep(x, wq, wk, wv, wo):
    """Build per-core input maps (layout transforms + bf16 casts only)."""
    import ml_dtypes
    bf16 = ml_dtypes.bfloat16

    x2 = np.ascontiguousarray(x.reshape(-1, HIDDEN))          # (16384, 2048)
    wqs = (wq / np.sqrt(np.float32(HD))).astype(np.float32)

    def wt4(w):   # [128, 16, 2048]: wt4[p, mt, kt*128+j] = w[mt*128+j, kt*128+p]
        return np.ascontiguousarray(
            w.reshape(H, 128, CT, 128).transpose(3, 0, 2, 1)
        ).reshape(128, H, CT * 128).astype(bf16)

    wq4, wk4, wv4, wo4 = wt4(wqs), wt4(wk), wt4(wv), wt4(wo)
    p = np.arange(128)[:, None]
    n = np.arange(128)[None, :]
    mask = np.where((p % GRP) == (n % GRP), 1.0, 0.0).astype(bf16)
    mask = np.tile(mask, (1, 4))
    ident = np.eye(128, dtype=np.float32).astype(bf16)

    in_maps = []
    for c in range(N_CORES):
        xs = x2[c * T_CORE:(c + 1) * T_CORE]                  # (2048, 2048)
        xtc = np.ascontiguousarray(
            xs.reshape(NTCH, TCH, CT, 128).transpose(3, 0, 2, 1)
        ).reshape(128, NTCH, CT * TCH).astype(bf16)
        in_maps.append({"xt": xtc, "wq4": wq4, "wk4": wk4, "wv4": wv4,
                        "wo4": wo4, "maskd": mask, "identd": ident})
    return in_maps


def kernel(x, wq, wk, wv, wo, inv_freq):
    # inv_freq is unused: RoPE is an identical orthogonal transform on q and k
    # at equal positions, and this attention only contracts same-position q·k,
    # so it cancels exactly.
    from concourse.bass_utils import run_bass_kernel_spmd

    x = np.asarray(x, dtype=np.float32)
    wq = np.asarray(wq, dtype=np.float32)
    wk = np.asarray(wk, dtype=np.float32)
    wv = np.asarray(wv, dtype=np.float32)
    wo = np.asarray(wo, dtype=np.float32)

    if "nc" not in _CACHED:
        _CACHED["nc"] = _build()
    nc = _CACHED["nc"]

    in_maps = _host_prep(x, wq, wk, wv, wo)
    res = run_bass_kernel_spmd(nc, in_maps, core_ids=list(range(N_CORES)))

    out = np.empty((N_CORES * T_CORE, HIDDEN), dtype=np.float32)
    for c in range(N_CORES):
        ot = np.asarray(res.results[c]["otb"]).astype(np.float32)  # (128,16,2048)
        out[c * T_CORE:(c + 1) * T_CORE] = (
            ot.transpose(2, 1, 0).reshape(T_CORE, HIDDEN))
    return out.reshape(x.shape[0], x.shape[1], HIDDEN)
